# revision 1
# baseline (speedup 1.0000x reference)
"""Abeles matrix (neutron reflectivity) kernel for 8 Trainium2 NeuronCores.

Algorithm (per point (b,q), L=64 layers):
  k_l = sqrt((q/2)^2 - 4*pi*(sld_l - sld_0)*1e-6 - i*4*pi*1e-9)  (stable branch form)
  r_l = Fresnel(k_l, k_{l+1}) * exp(-2 k_l k_{l+1} sigma_l^2)
  scaled transfer recurrence (u = v / prod(m00), layers descending):
      u0' = u0 + r*u1 ;  u1' = E*(r*u0 + u1),  E = e^{-2 t b}(cos 2ta - i sin 2ta)
  out = |u1/u0|^2

Sharding: pure data-parallel over batch, 32 rows of B=256 per core.
Per-core layout: 128 partitions = 32 b x 4 q-groups, 128 free = q within group.
"""
import sys
sys.path.insert(0, "/opt/trn_rl_repo")
import math
import numpy as np

import concourse.bass as bass
import concourse.mybir as mybir
from concourse import tile
from contextlib import ExitStack

AF = mybir.ActivationFunctionType
ALU = mybir.AluOpType
F32 = mybir.dt.float32
f32 = np.float32

B, Q, L = 256, 512, 64
NCORES = 8
BL = B // NCORES           # 32 batch rows per core
P = 128                    # partitions
QF = 128                   # q elements per partition
CHUNK = 10                 # layers per chunk
DEBUG_DUMP = False

YMAG = 4.0 * math.pi * 1e-9
Y2 = f32(YMAG * YMAG)
LNHALFY = f32(np.log(YMAG / 2.0))
PIO2 = f32(np.pi / 2.0)
INV2PI = f32(1.0 / (2.0 * np.pi))
MAGIC = f32(1.5 * 2.0 ** 23)
CW2PI_1 = f32(6.28125)                  # exact in f32
CW2PI_2 = f32(2.0 * np.pi - 6.28125)

# ---------------------------------------------------------------------------
# Toolchain workarounds for this walrus build:
# 1) InstDrain cannot carry sem waits -> re-emit as sync-engine wait_ge's.
# 2) TensorScalarPtr / Activation-with-AP-scale / CopyPredicated cannot carry
#    sem waits -> strip them onto same-engine wait_ge carrier instructions.
# ---------------------------------------------------------------------------
_PATCHED = False


def _install_patches():
    global _PATCHED
    if _PATCHED:
        return
    _PATCHED = True

    def _handles(tc):
        hm = {}
        for h in tc.sems.allocated().values():
            hm[h.name] = h
        return hm

    def _drain_and_barrier(self, tick_clock, wait_clock):
        nc = self.nc
        drain_inst = nc.sync.drain()
        wait_clock.add_sem_waits(
            drain_inst.ins, tile.ScopedClock({None: tick_clock.global_clock})
        )
        ii = drain_inst.ins
        si = ii.sync_info
        waits = list(si.on_wait) if si is not None else []
        if waits:
            ii.sync_info = mybir.SyncInfo(on_wait=[], on_update=list(si.on_update))
            hm = _handles(self)
            for w in waits:
                h = hm.get(w.ant_name)
                assert h is not None and w.wait_mode == "sem-ge-imm"
                nc.sync.wait_ge(h, w.wait_value)
        nc.all_engine_barrier()
        assert self.sems is not None
        popped = nc._tile_sem_poison_stack.pop()
        assert popped is self._sem_poison
        nc.clear_and_free_semaphores(list(self.sems.allocated().values()))
        nc.all_engine_barrier()

    tile.TileContext._drain_and_barrier = _drain_and_barrier

    _orig_commit = tile.TileContext._commit_instruction

    _KEEP1 = (mybir.InstTensorLoad, mybir.InstTensorSave, mybir.InstTensorCopy,
              mybir.InstTensorTensor)

    def _simple_aps(inst):
        # >2-dim APs lower to the S3S3D3-style structs with no wait slots
        try:
            for a in list(inst.ins) + list(inst.outs):
                ap = getattr(a, "ap", None)
                if ap is not None and len(ap) > 2:
                    return False
        except Exception:
            return False
        return True

    def _commit_instruction(self, inst, lazy_reg_writes=True):
        si = getattr(inst, "sync_info", None)
        if si is not None and si.on_wait:
            waits = list(si.on_wait)
            keep = []
            if isinstance(inst, _KEEP1) and _simple_aps(inst):
                # these structs tolerate one wait; strip the rest
                keep = waits[:1]
                waits = waits[1:]
            if waits:
                inst.sync_info = mybir.SyncInfo(on_wait=keep, on_update=list(si.on_update))
                hm = _handles(self)
                eng = self.nc.engines[inst.engine]
                for i in range(0, len(waits), 2):
                    grp = waits[i:i + 2]
                    h = hm.get(grp[0].ant_name)
                    assert h is not None and grp[0].wait_mode == "sem-ge-imm", grp
                    carrier = eng.wait_ge(h, grp[0].wait_value)
                    if len(grp) > 1:
                        csi = carrier.ins.sync_info
                        carrier.ins.sync_info = mybir.SyncInfo(
                            on_wait=list(grp),
                            on_update=list(csi.on_update) if csi else [])
        return _orig_commit(self, inst, lazy_reg_writes)

    tile.TileContext._commit_instruction = _commit_instruction


# ---------------------------------------------------------------------------
# Kernel builder (one NeuronCore program; SPMD across 8 cores)
# ---------------------------------------------------------------------------

def _build_kernel():
    _install_patches()
    nc = bass.Bass()

    d_qq = nc.declare_dram_parameter("qq", [P, QF], F32, isOutput=False)
    d_negc = nc.declare_dram_parameter("negc", [P, L + 1], F32, isOutput=False)
    d_dx = nc.declare_dram_parameter("dx", [P, L], F32, isOutput=False)
    d_s2m = nc.declare_dram_parameter("s2m", [P, L], F32, isOutput=False)
    d_s2p = nc.declare_dram_parameter("s2p", [P, L], F32, isOutput=False)
    d_t2 = nc.declare_dram_parameter("t2", [P, L], F32, isOutput=False)
    d_m2t = nc.declare_dram_parameter("m2t", [P, L], F32, isOutput=False)
    d_out = nc.declare_dram_parameter("out", [P, QF], F32, isOutput=True)
    d_dbg = {}
    if DEBUG_DUMP:
        for nm in ("RRE", "RIM", "ERE", "EIMP", "A", "B", "Rlev", "TA2d", "RATREd", "RATIMd", "S2Td", "C2Td", "SWd", "CWd"):
            w = CHUNK * QF if nm not in ("A", "B", "Rlev") else (CHUNK + 1) * QF
            d_dbg[nm] = nc.declare_dram_parameter("dbg_" + nm, [P, w], F32, isOutput=True)

    with tile.TileContext(nc) as tc, ExitStack() as ctx:
        pool = ctx.enter_context(tc.tile_pool(name="sb", bufs=1))

        def tl(name, shape, dtype=F32, bufs=1):
            return pool.tile(shape, dtype, tag=name, name=name, bufs=bufs)

        # persistent inputs
        qq = tl("qq", [P, QF])
        negc = tl("negc", [P, L + 1])
        dxp = tl("dxp", [P, L])
        s2m = tl("s2m", [P, L])
        s2p = tl("s2p", [P, L])
        t2 = tl("t2", [P, L])
        m2t = tl("m2t", [P, L])
        nc.sync.dma_start(qq[:], d_qq[:])
        nc.sync.dma_start(negc[:], d_negc[:])
        nc.sync.dma_start(dxp[:], d_dx[:])
        nc.sync.dma_start(s2m[:], d_s2m[:])
        nc.sync.dma_start(s2p[:], d_s2p[:])
        nc.sync.dma_start(t2[:], d_t2[:])
        nc.sync.dma_start(m2t[:], d_m2t[:])

        # constant bias vectors
        y2b = tl("y2b", [P, 1]);  nc.gpsimd.memset(y2b[:], float(Y2))
        lnhy = tl("lnhy", [P, 1]); nc.gpsimd.memset(lnhy[:], float(LNHALFY))
        pio2 = tl("pio2", [P, 1]); nc.gpsimd.memset(pio2[:], float(PIO2))

        # u state (ping-pong quads: blocks [u0r | u0i | u1r | u1i])
        ucur = tl("uq_a", [P, 4 * QF])
        unew = tl("uq_b", [P, 4 * QF])
        nc.gpsimd.memset(ucur[:, 0:QF], 1.0)
        nc.gpsimd.memset(ucur[:, QF:], 0.0)
        A4 = tl("A4", [P, 4 * QF]); B4 = tl("B4", [P, 4 * QF]); T4 = tl("T4", [P, 4 * QF])
        E4 = tl("E4", [P, 2 * QF]); E5 = tl("E5", [P, 2 * QF])

        def pbc(param, lv0, n_l):
            # [P, n_l, QF] view of param[:, lv0:lv0+n_l], value broadcast over q
            return param[:, lv0:lv0 + n_l].rearrange("p (l n) -> p l n", n=1).broadcast_to([P, n_l, QF])

        def blk(t_, n_l):
            return t_[:, :n_l * QF].rearrange("p (l n) -> p l n", n=QF)

        prev_sin = None          # last trig-set op of previous chunk
        CW = CHUNK * QF          # max chunk width (layers)
        CW1 = (CHUNK + 1) * QF   # max chunk width (levels)
        starts = list(range(0, L, CHUNK))
        chunks = [(s0_, min(CHUNK, L - s0_)) for s0_ in starts]

        for l0, CL in reversed(chunks):
            cw = CL * QF
            cw1 = (CL + 1) * QF
            # ---- k levels: X, MSK, R, A, B --------------------------------
            X = tl("X", [P, CW1])
            MSK = tl("MSK", [P, CW1], mybir.dt.uint32)
            R = tl("Rr", [P, CW1])
            SQU = tl("SQU", [P, CW1])   # SQ -> G
            UHB = tl("UHB", [P, CW1])   # AX -> H -> B
            UA = tl("UA", [P, CW1])     # U -> A
            T = tl("T", [P, CW1])
            G = SQU                      # SQ dead once R is computed
            qqbc = qq[:].rearrange("p (l n) -> p l n", l=1).broadcast_to([P, CL + 1, QF])
            nc.vector.tensor_add(blk(X, CL + 1), qqbc, pbc(negc, l0, CL + 1))
            nc.vector.tensor_scalar(MSK[:, :cw1], X[:, :cw1], 0.0, None, ALU.is_ge)
            nc.scalar.activation(SQU[:, :cw1], X[:, :cw1], AF.Square)                # SQ = X^2
            nc.scalar.activation(UHB[:, :cw1], X[:, :cw1], AF.Abs)                   # AX = |X|
            iR = nc.scalar.activation(R[:, :cw1], SQU[:, :cw1], AF.Sqrt, bias=y2b[:])  # R = sqrt(X^2+Y2)
            if prev_sin is not None:
                bass._add_dep_helper(iR.ins, prev_sin.ins, sync=False, reason="act-table-order")
            nc.vector.tensor_add(UA[:, :cw1], R[:, :cw1], UHB[:, :cw1])              # U = R + AX
            iT = nc.scalar.activation(T[:, :cw1], UA[:, :cw1], AF.Sqrt, bias=0.0, scale=0.5)  # T = sqrt(U/2)
            iH = nc.scalar.activation(UHB[:, :cw1], UA[:, :cw1], AF.Ln, bias=0.0, scale=0.5)   # H = ln(U/2)
            bass._add_dep_helper(iH.ins, iT.ins, sync=False, reason="act-table-order")
            nc.scalar.activation(G[:, :cw1], UHB[:, :cw1], AF.Exp, bias=lnhy[:], scale=-0.5)  # G=(Y/2)/T
            A = UA; Bt = UHB
            nc.scalar.copy(A[:, :cw1], G[:, :cw1])
            nc.vector.copy_predicated(A[:, :cw1], MSK[:, :cw1], T[:, :cw1])   # A = x>=0 ? T : G
            nc.scalar.copy(Bt[:, :cw1], T[:, :cw1])
            nc.vector.copy_predicated(Bt[:, :cw1], MSK[:, :cw1], G[:, :cw1])  # B = x>=0 ? G : T

            # ---- layer quantities ----------------------------------------
            ac, an = A[:, 0:cw], A[:, QF:cw1]
            bc, bn = Bt[:, 0:cw], Bt[:, QF:cw1]
            rc, rn = R[:, 0:cw], R[:, QF:cw1]
            xc, xn = X[:, 0:cw], X[:, QF:cw1]

            # 13 rotating CW-sized slots (lifetime-disjoint reuse)
            s = [tl(f"s{i}", [P, CW]) for i in range(13)]
            def V(t_):
                return t_[:, :cw]
            P1, P2, AB1, AB2 = s[0], s[1], s[2], s[3]
            nc.vector.tensor_mul(V(P1), ac, an)
            nc.vector.tensor_mul(V(P2), bc, bn)
            nc.vector.tensor_mul(V(AB1), ac, bn)
            nc.vector.tensor_mul(V(AB2), bc, an)
            PR, PP, PI, CI0 = s[4], s[5], s[6], s[7]
            nc.vector.tensor_sub(V(PR), V(P1), V(P2))
            nc.vector.tensor_add(V(PP), V(P1), V(P2))
            nc.vector.tensor_add(V(PI), V(AB1), V(AB2))
            nc.vector.tensor_sub(V(CI0), V(AB1), V(AB2))
            RSUM, SX = s[8], s[9]
            nc.vector.tensor_add(V(RSUM), rc, rn)
            nc.vector.tensor_add(V(SX), xc, xn)
            MAG = s[0]          # P1 dead
            nc.vector.scalar_tensor_tensor(V(MAG), V(PP), 2.0, V(RSUM), ALU.mult, ALU.add)
            LNM, LNR = s[1], s[2]   # P2, AB1 dead
            nc.scalar.activation(V(LNM), V(MAG), AF.Ln)
            nc.scalar.activation(V(LNR), V(RSUM), AF.Ln)
            ARG = s[3]          # AB2 dead
            NUM = s[0]          # MAG dead after LNM
            WI, TA2, TB2 = s[8], s[10], s[11]  # RSUM dead after LNR/MAG
            nc.vector.tensor_mul(blk(ARG, CL), blk(PR, CL), pbc(s2m, l0, CL))
            nc.vector.tensor_sub(V(ARG), V(ARG), V(LNM))
            nc.vector.tensor_mul(blk(NUM, CL), blk(SX, CL), pbc(dxp, l0, CL))
            nc.vector.tensor_mul(blk(WI, CL), blk(PI, CL), pbc(s2p, l0, CL))
            nc.vector.tensor_mul(blk(TA2, CL), blk(A, CL), pbc(t2, l0, CL))
            nc.vector.tensor_mul(blk(TB2, CL), blk(Bt, CL), pbc(m2t, l0, CL))
            # range-reduce TA2 (up to ~60 rad) into [-pi, pi] for the Sin table
            Ft = s[4]; KK = s[6]     # PR dead after ARG loop, PI dead after WI
            nc.vector.tensor_scalar(V(Ft), V(TA2), float(INV2PI), float(MAGIC), ALU.mult, ALU.add)
            nc.vector.tensor_scalar(V(KK), V(Ft), float(MAGIC), None, ALU.subtract)
            nc.vector.scalar_tensor_tensor(V(TA2), V(KK), float(-2.0 * np.pi), V(TA2), ALU.mult, ALU.add)
            ARG2 = s[5]          # PP dead after MAG
            nc.vector.tensor_sub(V(ARG2), V(ARG), V(LNR))
            RR, RR2, EE = s[1], s[2], s[6]   # LNM, LNR dead; PI dead after WI
            nc.scalar.activation(V(RR), V(ARG), AF.Exp)
            nc.scalar.activation(V(RR2), V(ARG2), AF.Exp)
            iEE = nc.scalar.activation(V(EE), V(TB2), AF.Exp)
            RATRE, RATIM = s[3], s[4]        # ARG dead after RR, PR dead after ARG loop
            nc.vector.tensor_mul(V(RATRE), V(NUM), V(RR2))
            nc.vector.scalar_tensor_tensor(V(RATIM), V(CI0), 2.0, V(RR), ALU.mult, ALU.mult)
            SW, CWt = s[0], s[5]             # NUM dead after RATRE, ARG2 dead after RR2
            S2T, C2T = s[7], s[12]           # CI0 dead after RATIM, ARG2 dead after RR2
            iSW = nc.scalar.activation(V(SW), V(WI), AF.Sin)
            bass._add_dep_helper(iSW.ins, iEE.ins, sync=False, reason="act-table-order")
            iCW = nc.scalar.activation(V(CWt), V(WI), AF.Sin, bias=pio2[:])
            bass._add_dep_helper(iCW.ins, iEE.ins, sync=False, reason="act-table-order")
            iS2 = nc.scalar.activation(V(S2T), V(TA2), AF.Sin)
            bass._add_dep_helper(iS2.ins, iEE.ins, sync=False, reason="act-table-order")
            # cos(TA2): shift by pi/2 then wrap args > pi back by 2*pi
            CSH = s[2]; CMSK = s[11]   # RR2 dead after RATRE, TB2 dead after EE
            nc.vector.tensor_scalar(V(CSH), V(TA2), float(PIO2), None, ALU.add)
            nc.vector.tensor_scalar(V(CMSK), V(CSH), float(np.pi), None, ALU.is_gt)
            nc.vector.scalar_tensor_tensor(V(CSH), V(CMSK), float(-2.0 * np.pi), V(CSH), ALU.mult, ALU.add)
            iC2 = nc.scalar.activation(V(C2T), V(CSH), AF.Sin)
            bass._add_dep_helper(iC2.ins, iEE.ins, sync=False, reason="act-table-order")
            prev_sin = iC2
            # r_nev = RAT * (cw + i sw) ; E = EE*(c2 - i s2)
            # RIMPM blocks per layer: [-r_im | +r_im]; EIMPM: [+E_imp | -E_imp]
            RRE = tl("RRE", [P, CW], bufs=2); RIMPM = tl("RIMPM", [P, 2 * CW], bufs=2)
            ERE = tl("ERE", [P, CW], bufs=2); EIMPM = tl("EIMPM", [P, 2 * CW], bufs=2)
            rimv = RIMPM[:, :2 * cw].rearrange("p (l t n) -> p l t n", t=2, n=QF)
            eimv = EIMPM[:, :2 * cw].rearrange("p (l t n) -> p l t n", t=2, n=QF)
            t1_, t2_ = s[8], s[9]            # WI dead after sins, SX dead after NUM
            nc.vector.tensor_mul(V(t1_), V(RATRE), V(CWt))
            nc.vector.tensor_mul(V(t2_), V(RATIM), V(SW))
            nc.vector.tensor_sub(V(RRE), V(t1_), V(t2_))
            nc.vector.tensor_mul(V(t1_), V(RATRE), V(SW))
            nc.vector.tensor_mul(V(t2_), V(RATIM), V(CWt))
            nc.vector.tensor_add(rimv[:, :, 1, :], blk(t1_, CL), blk(t2_, CL))
            nc.scalar.activation(rimv[:, :, 0, :], rimv[:, :, 1, :], AF.Copy, bias=0.0, scale=-1.0)
            nc.vector.tensor_mul(V(ERE), V(EE), V(C2T))
            nc.vector.tensor_mul(eimv[:, :, 0, :], blk(EE, CL), blk(S2T, CL))
            nc.scalar.activation(eimv[:, :, 1, :], eimv[:, :, 0, :], AF.Copy, bias=0.0, scale=-1.0)

            if DEBUG_DUMP and l0 == 0:
                for nm, tens in (("RRE", RRE), ("RIM", RIM), ("ERE", ERE), ("EIMP", EIMP),
                                 ("A", A), ("B", Bt), ("Rlev", R), ("TA2d", TA2),
                                 ("RATREd", RATRE), ("RATIMd", RATIM),
                                 ("S2Td", S2T), ("C2Td", C2T), ("SWd", SW), ("CWd", CWt)):
                    nc.sync.dma_start(d_dbg[nm][:, :tens.shape[1]], tens[:])
            # ---- sequential update over layers (descending) ---------------
            for j in range(CL - 1, -1, -1):
                sl = slice(j * QF, (j + 1) * QF)
                sl2 = slice(j * 2 * QF, (j + 1) * 2 * QF)
                U = ucur; Vq = unew
                U22 = U[:].rearrange("p (a b n) -> p a b n", a=2, b=2)
                u_rot = U22[:, ::-1, :, :]          # [u1r,u1i,u0r,u0i]
                u_rev = U22[:, ::-1, ::-1, :]       # [u1i,u1r,u0i,u0r]
                rre4 = RRE[:, sl].rearrange("p (a b n) -> p a b n", a=1, b=1).broadcast_to([P, 2, 2, QF])
                rim4 = RIMPM[:, sl2].rearrange("p (a t n) -> p a t n", a=1, t=2).broadcast_to([P, 2, 2, QF])
                nc.vector.tensor_mul(A4[:].rearrange("p (a b n) -> p a b n", a=2, b=2), u_rot, rre4)
                nc.vector.tensor_mul(B4[:].rearrange("p (a b n) -> p a b n", a=2, b=2), u_rev, rim4)
                nc.vector.tensor_add(T4[:], U[:], A4[:])
                nc.vector.tensor_add(Vq[:], T4[:], B4[:])   # [nu0r, nu0i, p1r, p1i]
                p1 = Vq[:, 2 * QF:4 * QF]
                p12 = p1.rearrange("p (b n) -> p b n", b=2)
                p1sw = p12[:, ::-1, :]
                ere2 = ERE[:, sl].rearrange("p (b n) -> p b n", b=1).broadcast_to([P, 2, QF])
                nc.vector.tensor_mul(E4[:].rearrange("p (b n) -> p b n", b=2), p12, ere2)
                nc.vector.tensor_mul(E5[:].rearrange("p (b n) -> p b n", b=2), p1sw,
                                     EIMPM[:, sl2].rearrange("p (b n) -> p b n", b=2))
                nc.vector.tensor_add(Vq[:, 2 * QF:4 * QF], E4[:], E5[:])
                ucur, unew = unew, ucur

        # ---- epilogue: out = |u1/u0|^2 -----------------------------------
        u0r, u0i = ucur[:, 0:QF], ucur[:, QF:2 * QF]
        u1r, u1i = ucur[:, 2 * QF:3 * QF], ucur[:, 3 * QF:4 * QF]
        d1 = tl("q1", [P, QF]); d2 = tl("q2", [P, QF]); d3 = tl("q3", [P, QF])
        nc.scalar.activation(d1[:], u0r, AF.Square)
        nc.scalar.activation(d2[:], u0i, AF.Square)
        nc.vector.tensor_add(d1[:], d1[:], d2[:])      # |u0|^2
        nc.scalar.activation(d2[:], d1[:], AF.Ln)
        nc.scalar.activation(d1[:], d2[:], AF.Exp, bias=0.0, scale=-1.0)  # 1/|u0|^2
        nc.vector.tensor_mul(d2[:], u1r, u0r)
        nc.vector.tensor_mul(d3[:], u1i, u0i)
        nc.vector.tensor_add(d2[:], d2[:], d3[:])
        nc.vector.tensor_mul(d2[:], d2[:], d1[:])      # qr
        OUT = tl("OUT", [P, QF])
        nc.vector.tensor_mul(d3[:], u1i, u0r)
        qi2 = tl("q4", [P, QF])
        nc.vector.tensor_mul(qi2[:], u1r, u0i)
        nc.vector.tensor_sub(d3[:], d3[:], qi2[:])
        nc.vector.tensor_mul(d3[:], d3[:], d1[:])      # qi
        nc.scalar.activation(d2[:], d2[:], AF.Square)
        nc.scalar.activation(d3[:], d3[:], AF.Square)
        nc.vector.tensor_add(OUT[:], d2[:], d3[:])
        nc.sync.dma_start(d_out[:], OUT[:])

    return nc


_NC_CACHE = None


def _get_nc():
    global _NC_CACHE
    if _NC_CACHE is None:
        _NC_CACHE = _build_kernel()
    return _NC_CACHE


def _prep_core_inputs(q, thickness, roughness, sld):
    """Host-side O(B*(Q+L)) prep; returns per-core input dicts."""
    q = q.astype(f32); th = thickness.astype(f32)
    rg = roughness.astype(f32); sld = sld.astype(f32)
    amb = sld[:, 0:1]
    negc64 = -(4.0 * math.pi * 1e-6) * (sld.astype(np.float64) - amb.astype(np.float64))
    negc = negc64.astype(f32)                       # [B, L+1]
    dx = (negc64[:, :-1] - negc64[:, 1:]).astype(f32)  # x_c - x_n, exact in f64
    s2m = (-2.0 * rg * rg).astype(f32)
    s2p = (2.0 * rg * rg).astype(f32)
    t2 = (2.0 * th).astype(f32)
    m2t = (-2.0 * th).astype(f32)
    qq = ((q * f32(0.5)) ** 2).astype(f32)          # [B, Q]

    def rep4(arr):  # [BL, K] -> [128, K] (each row repeated 4x)
        return np.repeat(arr, 4, axis=0).copy()

    in_maps = []
    for c in range(NCORES):
        bs = slice(c * BL, (c + 1) * BL)
        in_maps.append({
            "qq": qq[bs].reshape(P, QF).copy(),
            "negc": rep4(negc[bs]),
            "dx": rep4(dx[bs]),
            "s2m": rep4(s2m[bs]),
            "s2p": rep4(s2p[bs]),
            "t2": rep4(t2[bs]),
            "m2t": rep4(m2t[bs]),
        })
    return in_maps


def run(q, thickness, roughness, sld, trace=False, **trace_kwargs):
    from concourse.bass_utils import run_bass_kernel_spmd
    nc = _get_nc()
    in_maps = _prep_core_inputs(q, thickness, roughness, sld)
    res = run_bass_kernel_spmd(nc, in_maps, core_ids=list(range(NCORES)),
                               trace=trace, **trace_kwargs)
    out = np.empty((B, Q), f32)
    for c in range(NCORES):
        out[c * BL:(c + 1) * BL] = res.results[c]["out"].reshape(BL, Q)
    return out, res


def kernel(q, thickness, roughness, sld):
    out, _ = run(q, thickness, roughness, sld)
    return out



# revision 6
# speedup vs baseline: 1.0494x; 1.0494x over previous
"""Abeles matrix (neutron reflectivity) kernel for 8 Trainium2 NeuronCores.

Algorithm (per point (b,q), L=64 layers):
  k_l = A_l - i*B_l with A = sqrt((R+|x|)/2) branch-stable, B = (Y/2)/A,
        x = (q/2)^2 - 4pi*(sld_l - sld_0)*1e-6, Y = 4pi*1e-9, R = sqrt(x^2+Y^2)
  r_l = (k_c-k_n)/(k_c+k_n) * exp(-2 k_c k_n sigma^2)
      = (RDIF + 2i*CI0)/|S|^2 * exp(-2 s^2 PR) * e^{i WI},  WI = 2 s^2 PI <= ~0.02
      with small-angle e^{iWI} ~= (1 + i*WI)  (rel err <= WI^2/2 ~ 2e-4)
  E_l = exp(-2 t B) * (cos(2 t A) - i sin(2 t A))
  scaled transfer recurrence (fp16): u0' = u0 + r*u1 ; u1' = E*(r*u0 + u1)
  out = |u1/u0|^2

Sharding: pure data-parallel over batch, 32 rows of B=256 per core.
Per-core layout: 128 partitions = 32 b x 4 q-groups, 128 free = q within group.
Engines: DVE (fp16 2x where safe), Pool offload for independent fp32 chains,
ACT for transcendentals.
"""
import sys
sys.path.insert(0, "/opt/trn_rl_repo")
import math
import numpy as np

import concourse.bass as bass
import concourse.mybir as mybir
from concourse import tile
from contextlib import ExitStack

AF = mybir.ActivationFunctionType
ALU = mybir.AluOpType
F32 = mybir.dt.float32
F16 = mybir.dt.float16
f32 = np.float32

B, Q, L = 256, 512, 64
NCORES = 8
BL = B // NCORES           # 32 batch rows per core
P = 128                    # partitions
QF = 128                   # q elements per partition
CHUNK = 8                  # layers per chunk

YMAG = 4.0 * math.pi * 1e-9
Y2 = f32(YMAG * YMAG)
LNHALFY = f32(np.log(YMAG / 2.0))
PIO2 = f32(np.pi / 2.0)
INV2PI = f32(1.0 / (2.0 * np.pi))
MAGIC = f32(1.5 * 2.0 ** 23)

# ---------------------------------------------------------------------------
# Toolchain workarounds for this walrus build:
# 1) InstDrain cannot carry sem waits -> re-emit as sync-engine wait_ge's.
# 2) TensorScalarPtr / Activation-with-AP-scale / CopyPredicated cannot carry
#    sem waits -> strip them onto same-engine wait_ge carrier instructions.
# ---------------------------------------------------------------------------
_PATCHED = False


def _install_patches():
    global _PATCHED
    if _PATCHED:
        return
    _PATCHED = True

    def _handles(tc):
        hm = {}
        for h in tc.sems.allocated().values():
            hm[h.name] = h
        return hm

    def _drain_and_barrier(self, tick_clock, wait_clock):
        nc = self.nc
        drain_inst = nc.sync.drain()
        wait_clock.add_sem_waits(
            drain_inst.ins, tile.ScopedClock({None: tick_clock.global_clock})
        )
        ii = drain_inst.ins
        si = ii.sync_info
        waits = list(si.on_wait) if si is not None else []
        if waits:
            ii.sync_info = mybir.SyncInfo(on_wait=[], on_update=list(si.on_update))
            hm = _handles(self)
            for w in waits:
                h = hm.get(w.ant_name)
                assert h is not None and w.wait_mode == "sem-ge-imm"
                nc.sync.wait_ge(h, w.wait_value)
        nc.all_engine_barrier()
        assert self.sems is not None
        popped = nc._tile_sem_poison_stack.pop()
        assert popped is self._sem_poison
        nc.clear_and_free_semaphores(list(self.sems.allocated().values()))
        nc.all_engine_barrier()

    tile.TileContext._drain_and_barrier = _drain_and_barrier

    _orig_commit = tile.TileContext._commit_instruction

    _KEEP1 = (mybir.InstTensorLoad, mybir.InstTensorSave, mybir.InstTensorCopy,
              mybir.InstTensorTensor)

    def _simple_aps(inst):
        # >2-dim APs lower to the S3S3D3-style structs with no wait slots
        try:
            for a in list(inst.ins) + list(inst.outs):
                ap = getattr(a, "ap", None)
                if ap is not None and len(ap) > 2:
                    return False
        except Exception:
            return False
        return True

    def _commit_instruction(self, inst, lazy_reg_writes=True):
        si = getattr(inst, "sync_info", None)
        if si is not None and si.on_wait:
            waits = list(si.on_wait)
            keep = []
            if isinstance(inst, _KEEP1) and _simple_aps(inst):
                # these structs tolerate one wait; strip the rest
                keep = waits[:1]
                waits = waits[1:]
            if waits:
                inst.sync_info = mybir.SyncInfo(on_wait=keep, on_update=list(si.on_update))
                hm = _handles(self)
                eng = self.nc.engines[inst.engine]
                for i in range(0, len(waits), 2):
                    grp = waits[i:i + 2]
                    h = hm.get(grp[0].ant_name)
                    assert h is not None and grp[0].wait_mode == "sem-ge-imm", grp
                    carrier = eng.wait_ge(h, grp[0].wait_value)
                    if len(grp) > 1:
                        csi = carrier.ins.sync_info
                        carrier.ins.sync_info = mybir.SyncInfo(
                            on_wait=list(grp),
                            on_update=list(csi.on_update) if csi else [])
        return _orig_commit(self, inst, lazy_reg_writes)

    tile.TileContext._commit_instruction = _commit_instruction


# ---------------------------------------------------------------------------
# Kernel builder (one NeuronCore program; SPMD across 8 cores)
# ---------------------------------------------------------------------------

def _build_kernel():
    _install_patches()
    nc = bass.Bass()

    d_qq = nc.declare_dram_parameter("qq", [P, QF], F32, isOutput=False)
    d_negc = nc.declare_dram_parameter("negc", [P, L + 1], F32, isOutput=False)
    d_s2m = nc.declare_dram_parameter("s2m", [P, L], F32, isOutput=False)
    d_s2p = nc.declare_dram_parameter("s2p", [P, L], F16, isOutput=False)
    d_t2 = nc.declare_dram_parameter("t2", [P, L], F32, isOutput=False)
    d_m2t = nc.declare_dram_parameter("m2t", [P, L], F32, isOutput=False)
    d_out = nc.declare_dram_parameter("out", [P, QF], F32, isOutput=True)

    with tile.TileContext(nc) as tc, ExitStack() as ctx:
        pool = ctx.enter_context(tc.tile_pool(name="sb", bufs=1))

        def tl(name, shape, dtype=F32, bufs=1):
            return pool.tile(shape, dtype, tag=name, name=name, bufs=bufs)

        # persistent inputs
        qq = tl("qq", [P, QF])
        negc = tl("negc", [P, L + 1])
        s2m = tl("s2m", [P, L])
        s2p = tl("s2p", [P, L], F16)
        t2 = tl("t2", [P, L])
        m2t = tl("m2t", [P, L])
        nc.sync.dma_start(qq[:], d_qq[:])
        nc.sync.dma_start(negc[:], d_negc[:])
        nc.sync.dma_start(s2m[:], d_s2m[:])
        nc.sync.dma_start(s2p[:], d_s2p[:])
        nc.sync.dma_start(t2[:], d_t2[:])
        nc.sync.dma_start(m2t[:], d_m2t[:])

        # constant bias vectors
        y2b = tl("y2b", [P, 1]);  nc.gpsimd.memset(y2b[:], float(Y2))
        lnhy = tl("lnhy", [P, 1]); nc.gpsimd.memset(lnhy[:], float(LNHALFY))

        # u state (ping-pong quads: blocks [u0r | u0i | u1r | u1i]), fp16
        ucur = tl("uq_a", [P, 4 * QF], F16)
        unew = tl("uq_b", [P, 4 * QF], F16)
        nc.gpsimd.memset(ucur[:, 0:QF], 1.0)
        nc.gpsimd.memset(ucur[:, QF:], 0.0)
        A4 = tl("A4", [P, 4 * QF], F16); B4 = tl("B4", [P, 4 * QF], F16)
        T4 = tl("T4", [P, 4 * QF], F16)
        E4 = tl("E4", [P, 2 * QF], F16); E5 = tl("E5", [P, 2 * QF], F16)

        def pbc(param, lv0, n_l):
            # [P, n_l, QF] view of param[:, lv0:lv0+n_l], value broadcast over q
            return param[:, lv0:lv0 + n_l].rearrange("p (l n) -> p l n", n=1).broadcast_to([P, n_l, QF])

        def blk(t_, n_l):
            return t_[:, :n_l * QF].rearrange("p (l n) -> p l n", n=QF)

        prev_sin = None          # last trig-set op of previous chunk
        CW = CHUNK * QF          # max chunk width (layers)
        CW1 = (CHUNK + 1) * QF   # max chunk width (levels)
        starts = list(range(0, L, CHUNK))
        chunks = [(s0_, min(CHUNK, L - s0_)) for s0_ in starts]

        for l0, CL in reversed(chunks):
            cw = CL * QF
            cw1 = (CL + 1) * QF
            # ---- k levels: X, A, B ----------------------------------------
            # bufs=2 tiles: produced/consumed across the Pool/DVE boundary so
            # chunk c+1 Pool work can overlap chunk c DVE work.
            X = tl("X", [P, CW1], bufs=2)       # X -> later D1
            MSKf = tl("MSKf", [P, CW1], bufs=2)
            S1 = tl("S1", [P, CW1])             # SQ -> H -> CARG
            S2 = tl("S2", [P, CW1], bufs=2)     # AX -> TMG
            R = tl("R", [P, CW1], bufs=2)
            UA = tl("UA", [P, CW1], bufs=2)
            T = tl("T", [P, CW1], bufs=2)
            G = tl("G", [P, CW1], bufs=2)
            A = tl("A", [P, CW1], bufs=2)
            Bv = tl("Bv", [P, CW1], bufs=2)
            qqbc = qq[:].rearrange("p (l n) -> p l n", l=1).broadcast_to([P, CL + 1, QF])
            nc.gpsimd.tensor_add(blk(X, CL + 1), qqbc, pbc(negc, l0, CL + 1))
            nc.vector.tensor_scalar(MSKf[:, :cw1], X[:, :cw1], 0.0, None, ALU.is_ge)
            iSQ = nc.scalar.activation(S1[:, :cw1], X[:, :cw1], AF.Square)
            if prev_sin is not None:
                bass._add_dep_helper(iSQ.ins, prev_sin.ins, sync=False, reason="act-table-order")
            nc.scalar.activation(S2[:, :cw1], X[:, :cw1], AF.Abs)                 # AX
            nc.scalar.activation(R[:, :cw1], S1[:, :cw1], AF.Sqrt, bias=y2b[:])  # R = sqrt(X^2+Y2)
            nc.gpsimd.tensor_add(UA[:, :cw1], R[:, :cw1], S2[:, :cw1])           # UA = R + AX
            iT = nc.scalar.activation(T[:, :cw1], UA[:, :cw1], AF.Sqrt, bias=0.0, scale=0.5)
            iH = nc.scalar.activation(S1[:, :cw1], UA[:, :cw1], AF.Ln, bias=0.0, scale=0.5)
            bass._add_dep_helper(iH.ins, iT.ins, sync=False, reason="act-table-order")
            nc.scalar.activation(G[:, :cw1], S1[:, :cw1], AF.Exp, bias=lnhy[:], scale=-0.5)
            nc.gpsimd.tensor_sub(S2[:, :cw1], T[:, :cw1], G[:, :cw1])            # TMG = T-G
            nc.gpsimd.tensor_mul(X[:, :cw1], MSKf[:, :cw1], S2[:, :cw1])         # D1 = M*TMG (X dead)
            nc.gpsimd.tensor_add(A[:, :cw1], G[:, :cw1], X[:, :cw1])             # A = G + D1
            nc.gpsimd.tensor_sub(Bv[:, :cw1], T[:, :cw1], X[:, :cw1])            # B = T - D1

            ac, an = A[:, 0:cw], A[:, QF:cw1]
            bc, bn = Bv[:, 0:cw], Bv[:, QF:cw1]
            rc, rn = R[:, 0:cw], R[:, QF:cw1]

            # ---- layer quantities -----------------------------------------
            P1 = tl("P1", [P, CW])              # P1 -> LNM
            AB1 = tl("AB1", [P, CW])            # AB1 -> ARG
            PR = tl("PR", [P, CW])              # PR -> TB2
            PP = tl("PP", [P, CW])
            CI0 = tl("CI0", [P, CW])
            MAG = tl("MAG", [P, CW])            # MAG -> RR
            TA2 = tl("TA2", [P, CW])
            Ft = tl("Ft", [P, CW])
            KK = tl("KK", [P, CW])
            P2 = tl("P2", [P, CW], bufs=2)
            AB2 = tl("AB2", [P, CW], bufs=2)
            RSUM = tl("RSUM", [P, CW], bufs=2)
            RDIF = tl("RDIF", [P, CW], bufs=2)
            PI16 = tl("PI16", [P, CW], F16)
            RATRE = tl("RATRE", [P, CW], F16)
            RATIM = tl("RATIM", [P, CW], F16)
            WI16 = tl("WI16", [P, CW], F16)     # WI -> C2T
            tA16 = tl("tA16", [P, CW], F16)     # tA -> S2T
            tB16 = tl("tB16", [P, CW], F16)     # tB -> EE

            nc.vector.tensor_mul(P1[:, :cw], ac, an)
            nc.gpsimd.tensor_mul(P2[:, :cw], bc, bn)
            nc.vector.tensor_mul(AB1[:, :cw], ac, bn)
            nc.gpsimd.tensor_mul(AB2[:, :cw], bc, an)
            nc.gpsimd.tensor_add(RSUM[:, :cw], rc, rn)
            nc.gpsimd.tensor_sub(RDIF[:, :cw], rc, rn)
            nc.vector.tensor_sub(PR[:, :cw], P1[:, :cw], P2[:, :cw])
            nc.vector.tensor_add(PP[:, :cw], P1[:, :cw], P2[:, :cw])
            nc.vector.tensor_sub(CI0[:, :cw], AB1[:, :cw], AB2[:, :cw])
            nc.vector.tensor_add(PI16[:, :cw], AB1[:, :cw], AB2[:, :cw])
            nc.vector.scalar_tensor_tensor(MAG[:, :cw], PP[:, :cw], 2.0, RSUM[:, :cw], ALU.mult, ALU.add)
            iLNM = nc.scalar.activation(P1[:, :cw], MAG[:, :cw], AF.Ln)          # LNM (P1 dead)
            nc.vector.tensor_mul(blk(AB1, CL), blk(PR, CL), pbc(s2m, l0, CL))    # ARG = PR*(-2s^2)
            nc.vector.tensor_sub(AB1[:, :cw], AB1[:, :cw], P1[:, :cw])           # ARG -= LNM
            iRR = nc.scalar.activation(MAG[:, :cw], AB1[:, :cw], AF.Exp)         # RR (MAG dead)
            nc.vector.tensor_mul(RATRE[:, :cw], RDIF[:, :cw], MAG[:, :cw])
            nc.vector.scalar_tensor_tensor(RATIM[:, :cw], CI0[:, :cw], 2.0, MAG[:, :cw], ALU.mult, ALU.mult)
            nc.vector.tensor_mul(blk(WI16, CL), blk(PI16, CL), pbc(s2p, l0, CL))  # WI = PI*2s^2
            # r = (RATRE + i RATIM)(1 + i WI): small-angle phase
            RRE = tl("RRE", [P, CW], F16, bufs=2)
            RIMPM = tl("RIMPM", [P, 2 * CW], F16, bufs=2)
            ERE = tl("ERE", [P, CW], F16, bufs=2)
            EIMPM = tl("EIMPM", [P, 2 * CW], F16, bufs=2)
            rimv = RIMPM[:, :2 * cw].rearrange("p (l t n) -> p l t n", t=2, n=QF)
            eimv = EIMPM[:, :2 * cw].rearrange("p (l t n) -> p l t n", t=2, n=QF)
            nc.vector.tensor_mul(tA16[:, :cw], RATIM[:, :cw], WI16[:, :cw])
            nc.vector.tensor_sub(RRE[:, :cw], RATRE[:, :cw], tA16[:, :cw])
            nc.vector.tensor_mul(tB16[:, :cw], RATRE[:, :cw], WI16[:, :cw])
            nc.vector.tensor_add(rimv[:, :, 1, :], blk(tB16, CL), blk(RATIM, CL))
            nc.scalar.activation(rimv[:, :, 0, :], rimv[:, :, 1, :], AF.Copy, bias=0.0, scale=-1.0)
            # ---- E = exp(-2tB)*(cos 2tA - i sin 2tA) ----------------------
            nc.vector.tensor_mul(blk(TA2, CL), blk(A, CL), pbc(t2, l0, CL))
            nc.vector.tensor_mul(blk(PR, CL), blk(Bv, CL), pbc(m2t, l0, CL))     # TB2 (PR dead)
            nc.vector.tensor_scalar(Ft[:, :cw], TA2[:, :cw], float(INV2PI), float(MAGIC), ALU.mult, ALU.add)
            nc.vector.tensor_scalar(KK[:, :cw], Ft[:, :cw], float(MAGIC), None, ALU.subtract)
            nc.vector.scalar_tensor_tensor(TA2[:, :cw], KK[:, :cw], float(-2.0 * np.pi), TA2[:, :cw], ALU.mult, ALU.add)
            # CARG = wrap(TA2r + pi/2) into (-pi, pi]: S1 dead, Ft dead (->CMSK)
            nc.vector.tensor_scalar(S1[:, :cw], TA2[:, :cw], float(PIO2), None, ALU.add)
            nc.vector.tensor_scalar(Ft[:, :cw], S1[:, :cw], float(np.pi), None, ALU.is_gt)
            nc.vector.scalar_tensor_tensor(S1[:, :cw], Ft[:, :cw], float(-2.0 * np.pi), S1[:, :cw], ALU.mult, ALU.add)
            iEE = nc.scalar.activation(tB16[:, :cw], PR[:, :cw], AF.Exp)         # EE fp16 (tB dead)
            iS2 = nc.scalar.activation(tA16[:, :cw], TA2[:, :cw], AF.Sin)        # S2T fp16 (tA dead)
            bass._add_dep_helper(iS2.ins, iEE.ins, sync=False, reason="act-table-order")
            iC2 = nc.scalar.activation(WI16[:, :cw], S1[:, :cw], AF.Sin)         # C2T fp16 (WI dead)
            bass._add_dep_helper(iC2.ins, iEE.ins, sync=False, reason="act-table-order")
            prev_sin = iC2
            nc.vector.tensor_mul(ERE[:, :cw], tB16[:, :cw], WI16[:, :cw])        # ERE = EE*C2T
            nc.vector.tensor_mul(eimv[:, :, 0, :], blk(tB16, CL), blk(tA16, CL))  # +EIM = EE*S2T
            nc.scalar.activation(eimv[:, :, 1, :], eimv[:, :, 0, :], AF.Copy, bias=0.0, scale=-1.0)

            # ---- sequential update over layers (descending), fp16 ---------
            for j in range(CL - 1, -1, -1):
                sl = slice(j * QF, (j + 1) * QF)
                sl2 = slice(j * 2 * QF, (j + 1) * 2 * QF)
                U = ucur; Vq = unew
                U22 = U[:].rearrange("p (a b n) -> p a b n", a=2, b=2)
                u_rot = U22[:, ::-1, :, :]          # [u1r,u1i,u0r,u0i]
                u_rev = U22[:, ::-1, ::-1, :]       # [u1i,u1r,u0i,u0r]
                rre4 = RRE[:, sl].rearrange("p (a b n) -> p a b n", a=1, b=1).broadcast_to([P, 2, 2, QF])
                rim4 = RIMPM[:, sl2].rearrange("p (a t n) -> p a t n", a=1, t=2).broadcast_to([P, 2, 2, QF])
                nc.vector.tensor_mul(A4[:].rearrange("p (a b n) -> p a b n", a=2, b=2), u_rot, rre4)
                nc.vector.tensor_mul(B4[:].rearrange("p (a b n) -> p a b n", a=2, b=2), u_rev, rim4)
                nc.vector.tensor_add(T4[:], U[:], A4[:])
                nc.vector.tensor_add(Vq[:], T4[:], B4[:])   # [nu0r, nu0i, p1r, p1i]
                p1 = Vq[:, 2 * QF:4 * QF]
                p12 = p1.rearrange("p (b n) -> p b n", b=2)
                p1sw = p12[:, ::-1, :]
                ere2 = ERE[:, sl].rearrange("p (b n) -> p b n", b=1).broadcast_to([P, 2, QF])
                nc.vector.tensor_mul(E4[:].rearrange("p (b n) -> p b n", b=2), p12, ere2)
                nc.vector.tensor_mul(E5[:].rearrange("p (b n) -> p b n", b=2), p1sw,
                                     EIMPM[:, sl2].rearrange("p (b n) -> p b n", b=2))
                nc.vector.tensor_add(Vq[:, 2 * QF:4 * QF], E4[:], E5[:])
                ucur, unew = unew, ucur

        # ---- epilogue: out = |u1/u0|^2 -----------------------------------
        u0r, u0i = ucur[:, 0:QF], ucur[:, QF:2 * QF]
        u1r, u1i = ucur[:, 2 * QF:3 * QF], ucur[:, 3 * QF:4 * QF]
        d1 = tl("q1", [P, QF]); d2 = tl("q2", [P, QF]); d3 = tl("q3", [P, QF])
        nc.scalar.activation(d1[:], u0r, AF.Square)
        nc.scalar.activation(d2[:], u0i, AF.Square)
        nc.vector.tensor_add(d1[:], d1[:], d2[:])      # |u0|^2
        nc.scalar.activation(d2[:], d1[:], AF.Ln)
        nc.scalar.activation(d1[:], d2[:], AF.Exp, bias=0.0, scale=-1.0)  # 1/|u0|^2
        nc.vector.tensor_mul(d2[:], u1r, u0r)
        nc.vector.tensor_mul(d3[:], u1i, u0i)
        nc.vector.tensor_add(d2[:], d2[:], d3[:])
        nc.vector.tensor_mul(d2[:], d2[:], d1[:])      # qr
        OUT = tl("OUT", [P, QF])
        nc.vector.tensor_mul(d3[:], u1i, u0r)
        qi2 = tl("q4", [P, QF])
        nc.vector.tensor_mul(qi2[:], u1r, u0i)
        nc.vector.tensor_sub(d3[:], d3[:], qi2[:])
        nc.vector.tensor_mul(d3[:], d3[:], d1[:])      # qi
        nc.scalar.activation(d2[:], d2[:], AF.Square)
        nc.scalar.activation(d3[:], d3[:], AF.Square)
        nc.vector.tensor_add(OUT[:], d2[:], d3[:])
        nc.sync.dma_start(d_out[:], OUT[:])

    return nc


_NC_CACHE = None


def _get_nc():
    global _NC_CACHE
    if _NC_CACHE is None:
        _NC_CACHE = _build_kernel()
    return _NC_CACHE


def _prep_core_inputs(q, thickness, roughness, sld):
    """Host-side O(B*(Q+L)) prep; returns per-core input dicts."""
    q = q.astype(f32); th = thickness.astype(f32)
    rg = roughness.astype(f32); sld = sld.astype(f32)
    amb = sld[:, 0:1]
    negc64 = -(4.0 * math.pi * 1e-6) * (sld.astype(np.float64) - amb.astype(np.float64))
    negc = negc64.astype(f32)                       # [B, L+1]
    s2m = (-2.0 * rg * rg).astype(f32)
    s2p = (2.0 * rg * rg).astype(np.float16)
    t2 = (2.0 * th).astype(f32)
    m2t = (-2.0 * th).astype(f32)
    qq = ((q * f32(0.5)) ** 2).astype(f32)          # [B, Q]

    def rep4(arr):  # [BL, K] -> [128, K] (each row repeated 4x)
        return np.repeat(arr, 4, axis=0).copy()

    in_maps = []
    for c in range(NCORES):
        bs = slice(c * BL, (c + 1) * BL)
        in_maps.append({
            "qq": qq[bs].reshape(P, QF).copy(),
            "negc": rep4(negc[bs]),
            "s2m": rep4(s2m[bs]),
            "s2p": rep4(s2p[bs]),
            "t2": rep4(t2[bs]),
            "m2t": rep4(m2t[bs]),
        })
    return in_maps


def run(q, thickness, roughness, sld, trace=False, **trace_kwargs):
    from concourse.bass_utils import run_bass_kernel_spmd
    nc = _get_nc()
    in_maps = _prep_core_inputs(q, thickness, roughness, sld)
    res = run_bass_kernel_spmd(nc, in_maps, core_ids=list(range(NCORES)),
                               trace=trace, **trace_kwargs)
    out = np.empty((B, Q), f32)
    for c in range(NCORES):
        out[c * BL:(c + 1) * BL] = res.results[c]["out"].reshape(BL, Q)
    return out, res


def kernel(q, thickness, roughness, sld):
    out, _ = run(q, thickness, roughness, sld)
    return out


# revision 7
# speedup vs baseline: 1.3168x; 1.2549x over previous
"""Abeles matrix (neutron reflectivity) kernel for 8 Trainium2 NeuronCores.

Algorithm (per point (b,q), L=64 layers):
  k_l = A_l - i*B_l with A = sqrt((R+|x|)/2) branch-stable, B = (Y/2)/A,
        x = (q/2)^2 - 4pi*(sld_l - sld_0)*1e-6, Y = 4pi*1e-9, R = sqrt(x^2+Y^2)
  r_l = (RDIF + 2i*CI0)/|S|^2 * exp(-2 s^2 PR) * (1 + i*WI)   [small-angle phase,
        WI = 2 s^2 PI <= ~0.02 rad, rel err <= WI^2/2 ~ 2e-4]
  E_l = exp(-2 t B) * (cos(2 t A) - i sin(2 t A)),  cos(x) = sin(pi/2 - |x|)
  scaled transfer recurrence (fp16): u0' = u0 + r*u1 ; u1' = E*(r*u0 + u1)
  out = |u1/u0|^2

Sharding: pure data-parallel over batch, 32 rows of B=256 per core.
Per-core layout: 128 partitions = 32 b x 4 q-groups, 128 free = q within group.
All elementwise on DVE (fp16 2x where precision allows), transcendentals on ACT.
"""
import sys
sys.path.insert(0, "/opt/trn_rl_repo")
import math
import numpy as np

import concourse.bass as bass
import concourse.mybir as mybir
from concourse import tile
from contextlib import ExitStack

AF = mybir.ActivationFunctionType
ALU = mybir.AluOpType
F32 = mybir.dt.float32
F16 = mybir.dt.float16
f32 = np.float32

B, Q, L = 256, 512, 64
NCORES = 8
BL = B // NCORES           # 32 batch rows per core
P = 128                    # partitions
QF = 128                   # q elements per partition
CHUNK = 10                 # layers per chunk

YMAG = 4.0 * math.pi * 1e-9
Y2 = f32(YMAG * YMAG)
LNHALFY = f32(np.log(YMAG / 2.0))
PIO2 = f32(np.pi / 2.0)
INV2PI = f32(1.0 / (2.0 * np.pi))
MAGIC = f32(1.5 * 2.0 ** 23)

# ---------------------------------------------------------------------------
# Toolchain workarounds for this walrus build:
# 1) InstDrain cannot carry sem waits -> re-emit as sync-engine wait_ge's.
# 2) TensorScalarPtr / Activation-with-AP-scale / CopyPredicated cannot carry
#    sem waits -> strip them onto same-engine wait_ge carrier instructions.
# ---------------------------------------------------------------------------
_PATCHED = False


def _install_patches():
    global _PATCHED
    if _PATCHED:
        return
    _PATCHED = True

    def _handles(tc):
        hm = {}
        for h in tc.sems.allocated().values():
            hm[h.name] = h
        return hm

    def _drain_and_barrier(self, tick_clock, wait_clock):
        nc = self.nc
        drain_inst = nc.sync.drain()
        wait_clock.add_sem_waits(
            drain_inst.ins, tile.ScopedClock({None: tick_clock.global_clock})
        )
        ii = drain_inst.ins
        si = ii.sync_info
        waits = list(si.on_wait) if si is not None else []
        if waits:
            ii.sync_info = mybir.SyncInfo(on_wait=[], on_update=list(si.on_update))
            hm = _handles(self)
            for w in waits:
                h = hm.get(w.ant_name)
                assert h is not None and w.wait_mode == "sem-ge-imm"
                nc.sync.wait_ge(h, w.wait_value)
        nc.all_engine_barrier()
        assert self.sems is not None
        popped = nc._tile_sem_poison_stack.pop()
        assert popped is self._sem_poison
        nc.clear_and_free_semaphores(list(self.sems.allocated().values()))
        nc.all_engine_barrier()

    tile.TileContext._drain_and_barrier = _drain_and_barrier

    _orig_commit = tile.TileContext._commit_instruction

    _KEEP1 = (mybir.InstTensorLoad, mybir.InstTensorSave, mybir.InstTensorCopy,
              mybir.InstTensorTensor)

    def _simple_aps(inst):
        # >2-dim APs lower to the S3S3D3-style structs with no wait slots
        try:
            for a in list(inst.ins) + list(inst.outs):
                ap = getattr(a, "ap", None)
                if ap is not None and len(ap) > 2:
                    return False
        except Exception:
            return False
        return True

    def _commit_instruction(self, inst, lazy_reg_writes=True):
        si = getattr(inst, "sync_info", None)
        if si is not None and si.on_wait:
            waits = list(si.on_wait)
            keep = []
            if isinstance(inst, _KEEP1) and _simple_aps(inst):
                # these structs tolerate one wait; strip the rest
                keep = waits[:1]
                waits = waits[1:]
            if waits:
                inst.sync_info = mybir.SyncInfo(on_wait=keep, on_update=list(si.on_update))
                hm = _handles(self)
                eng = self.nc.engines[inst.engine]
                for i in range(0, len(waits), 2):
                    grp = waits[i:i + 2]
                    h = hm.get(grp[0].ant_name)
                    assert h is not None and grp[0].wait_mode == "sem-ge-imm", grp
                    carrier = eng.wait_ge(h, grp[0].wait_value)
                    if len(grp) > 1:
                        csi = carrier.ins.sync_info
                        carrier.ins.sync_info = mybir.SyncInfo(
                            on_wait=list(grp),
                            on_update=list(csi.on_update) if csi else [])
        return _orig_commit(self, inst, lazy_reg_writes)

    tile.TileContext._commit_instruction = _commit_instruction


# ---------------------------------------------------------------------------
# Kernel builder (one NeuronCore program; SPMD across 8 cores)
# ---------------------------------------------------------------------------

def _build_kernel():
    _install_patches()
    nc = bass.Bass()

    d_qq = nc.declare_dram_parameter("qq", [P, QF], F32, isOutput=False)
    d_negc = nc.declare_dram_parameter("negc", [P, L + 1], F32, isOutput=False)
    d_s2m = nc.declare_dram_parameter("s2m", [P, L], F32, isOutput=False)
    d_s2p = nc.declare_dram_parameter("s2p", [P, L], F16, isOutput=False)
    d_t2 = nc.declare_dram_parameter("t2", [P, L], F32, isOutput=False)
    d_m2t = nc.declare_dram_parameter("m2t", [P, L], F32, isOutput=False)
    d_out = nc.declare_dram_parameter("out", [P, QF], F32, isOutput=True)

    with tile.TileContext(nc) as tc, ExitStack() as ctx:
        pool = ctx.enter_context(tc.tile_pool(name="sb", bufs=1))

        def tl(name, shape, dtype=F32, bufs=1):
            return pool.tile(shape, dtype, tag=name, name=name, bufs=bufs)

        # persistent inputs
        qq = tl("qq", [P, QF])
        negc = tl("negc", [P, L + 1])
        s2m = tl("s2m", [P, L])
        s2p = tl("s2p", [P, L], F16)
        t2 = tl("t2", [P, L])
        m2t = tl("m2t", [P, L])
        nc.sync.dma_start(qq[:], d_qq[:])
        nc.sync.dma_start(negc[:], d_negc[:])
        nc.sync.dma_start(s2m[:], d_s2m[:])
        nc.sync.dma_start(s2p[:], d_s2p[:])
        nc.sync.dma_start(t2[:], d_t2[:])
        nc.sync.dma_start(m2t[:], d_m2t[:])

        # constant bias vectors
        y2b = tl("y2b", [P, 1]);  nc.gpsimd.memset(y2b[:], float(Y2))
        lnhy = tl("lnhy", [P, 1]); nc.gpsimd.memset(lnhy[:], float(LNHALFY))

        # u state (ping-pong quads: blocks [u0r | u0i | u1r | u1i]), fp16
        ucur = tl("uq_a", [P, 4 * QF], F16)
        unew = tl("uq_b", [P, 4 * QF], F16)
        nc.gpsimd.memset(ucur[:, 0:QF], 1.0)
        nc.gpsimd.memset(ucur[:, QF:], 0.0)
        A4 = tl("A4", [P, 4 * QF], F16); B4 = tl("B4", [P, 4 * QF], F16)
        T4 = tl("T4", [P, 4 * QF], F16)
        E4 = tl("E4", [P, 2 * QF], F16); E5 = tl("E5", [P, 2 * QF], F16)

        def pbc(param, lv0, n_l):
            # [P, n_l, QF] view of param[:, lv0:lv0+n_l], value broadcast over q
            return param[:, lv0:lv0 + n_l].rearrange("p (l n) -> p l n", n=1).broadcast_to([P, n_l, QF])

        def blk(t_, n_l):
            return t_[:, :n_l * QF].rearrange("p (l n) -> p l n", n=QF)

        prev_sin = None          # last trig-set op of previous chunk
        CW = CHUNK * QF          # max chunk width (layers)
        CW1 = (CHUNK + 1) * QF   # max chunk width (levels)
        starts = list(range(0, L, CHUNK))
        chunks = [(s0_, min(CHUNK, L - s0_)) for s0_ in starts]

        for l0, CL in reversed(chunks):
            cw = CL * QF
            cw1 = (CL + 1) * QF
            # ---- k levels: X, MSK, R, A, B --------------------------------
            X = tl("X", [P, CW1])
            MSK = tl("MSK", [P, CW1], mybir.dt.uint32)
            R = tl("Rr", [P, CW1])
            SQU = tl("SQU", [P, CW1])   # SQ -> G
            UHB = tl("UHB", [P, CW1])   # AX -> H -> B
            UA = tl("UA", [P, CW1])     # U -> A
            T = tl("T", [P, CW1])
            G = SQU                      # SQ dead once R is computed
            qqbc = qq[:].rearrange("p (l n) -> p l n", l=1).broadcast_to([P, CL + 1, QF])
            nc.vector.tensor_add(blk(X, CL + 1), qqbc, pbc(negc, l0, CL + 1))
            nc.vector.tensor_scalar(MSK[:, :cw1], X[:, :cw1], 0.0, None, ALU.is_ge)
            iSQ = nc.scalar.activation(SQU[:, :cw1], X[:, :cw1], AF.Square)             # SQ = X^2
            if prev_sin is not None:
                bass._add_dep_helper(iSQ.ins, prev_sin.ins, sync=False, reason="act-table-order")
            nc.scalar.activation(UHB[:, :cw1], X[:, :cw1], AF.Abs)                      # AX = |X|
            nc.scalar.activation(R[:, :cw1], SQU[:, :cw1], AF.Sqrt, bias=y2b[:])        # R = sqrt(X^2+Y2)
            nc.vector.tensor_add(UA[:, :cw1], R[:, :cw1], UHB[:, :cw1])                 # U = R + AX
            iT = nc.scalar.activation(T[:, :cw1], UA[:, :cw1], AF.Sqrt, bias=0.0, scale=0.5)  # T = sqrt(U/2)
            iH = nc.scalar.activation(UHB[:, :cw1], UA[:, :cw1], AF.Ln, bias=0.0, scale=0.5)   # H = ln(U/2)
            bass._add_dep_helper(iH.ins, iT.ins, sync=False, reason="act-table-order")
            nc.scalar.activation(G[:, :cw1], UHB[:, :cw1], AF.Exp, bias=lnhy[:], scale=-0.5)  # G=(Y/2)/T
            A = UA; Bt = UHB
            nc.scalar.copy(A[:, :cw1], G[:, :cw1])
            nc.vector.copy_predicated(A[:, :cw1], MSK[:, :cw1], T[:, :cw1])   # A = x>=0 ? T : G
            nc.scalar.copy(Bt[:, :cw1], T[:, :cw1])
            nc.vector.copy_predicated(Bt[:, :cw1], MSK[:, :cw1], G[:, :cw1])  # B = x>=0 ? G : T

            ac, an = A[:, 0:cw], A[:, QF:cw1]
            bc, bn = Bt[:, 0:cw], Bt[:, QF:cw1]
            rc, rn = R[:, 0:cw], R[:, QF:cw1]

            # ---- layer quantities (fp32 where cancellation matters) -------
            P1 = tl("P1", [P, CW])              # P1 -> LNM
            P2 = tl("P2", [P, CW])              # P2 -> TA2
            AB1 = tl("AB1", [P, CW])            # AB1 -> ARG
            AB2 = tl("AB2", [P, CW])            # AB2 -> TB2
            PR = tl("PR", [P, CW])              # PR -> Ft
            PP = tl("PP", [P, CW])              # PP -> KK
            CI0 = tl("CI0", [P, CW])            # CI0 -> |TA2r|
            RSUM = tl("RSUM", [P, CW])
            RDIF = tl("RDIF", [P, CW])          # RDIF -> CARG
            MAG = tl("MAG", [P, CW])            # MAG -> RR
            PI16 = tl("PI16", [P, CW], F16)
            RATRE = tl("RATRE", [P, CW], F16)
            RATIM = tl("RATIM", [P, CW], F16)
            WI16 = tl("WI16", [P, CW], F16)     # WI -> C2T
            tA16 = tl("tA16", [P, CW], F16)     # tA -> S2T
            tB16 = tl("tB16", [P, CW], F16)     # tB -> EE

            nc.vector.tensor_mul(P1[:, :cw], ac, an)
            nc.vector.tensor_mul(P2[:, :cw], bc, bn)
            nc.vector.tensor_mul(AB1[:, :cw], ac, bn)
            nc.vector.tensor_mul(AB2[:, :cw], bc, an)
            nc.vector.tensor_add(RSUM[:, :cw], rc, rn)
            nc.vector.tensor_sub(RDIF[:, :cw], rc, rn)
            nc.vector.tensor_sub(PR[:, :cw], P1[:, :cw], P2[:, :cw])
            nc.vector.tensor_add(PP[:, :cw], P1[:, :cw], P2[:, :cw])
            nc.vector.tensor_sub(CI0[:, :cw], AB1[:, :cw], AB2[:, :cw])
            nc.vector.tensor_add(PI16[:, :cw], AB1[:, :cw], AB2[:, :cw])
            nc.vector.scalar_tensor_tensor(MAG[:, :cw], PP[:, :cw], 2.0, RSUM[:, :cw], ALU.mult, ALU.add)
            nc.scalar.activation(P1[:, :cw], MAG[:, :cw], AF.Ln)                 # LNM (P1 dead)
            nc.vector.tensor_mul(blk(AB1, CL), blk(PR, CL), pbc(s2m, l0, CL))    # ARG = PR*(-2s^2)
            nc.vector.tensor_sub(AB1[:, :cw], AB1[:, :cw], P1[:, :cw])           # ARG -= LNM
            nc.scalar.activation(MAG[:, :cw], AB1[:, :cw], AF.Exp)               # RR (MAG dead)
            nc.vector.tensor_mul(RATRE[:, :cw], RDIF[:, :cw], MAG[:, :cw])
            nc.vector.scalar_tensor_tensor(RATIM[:, :cw], CI0[:, :cw], 2.0, MAG[:, :cw], ALU.mult, ALU.mult)
            nc.vector.tensor_mul(blk(WI16, CL), blk(PI16, CL), pbc(s2p, l0, CL))  # WI = PI*2s^2
            # r = (RATRE + i RATIM)(1 + i WI): small-angle phase
            RRE = tl("RRE", [P, CW], F16, bufs=2)
            RIMPM = tl("RIMPM", [P, 2 * CW], F16, bufs=2)
            ERE = tl("ERE", [P, CW], F16, bufs=2)
            EIMPM = tl("EIMPM", [P, 2 * CW], F16, bufs=2)
            rimv = RIMPM[:, :2 * cw].rearrange("p (l t n) -> p l t n", t=2, n=QF)
            eimv = EIMPM[:, :2 * cw].rearrange("p (l t n) -> p l t n", t=2, n=QF)
            nc.vector.tensor_mul(tA16[:, :cw], RATIM[:, :cw], WI16[:, :cw])
            nc.vector.tensor_sub(RRE[:, :cw], RATRE[:, :cw], tA16[:, :cw])
            nc.vector.tensor_mul(tB16[:, :cw], RATRE[:, :cw], WI16[:, :cw])
            nc.vector.tensor_add(rimv[:, :, 1, :], blk(tB16, CL), blk(RATIM, CL))
            nc.scalar.activation(rimv[:, :, 0, :], rimv[:, :, 1, :], AF.Copy, bias=0.0, scale=-1.0)
            # ---- E = exp(-2tB)*(cos 2tA - i sin 2tA) ----------------------
            nc.vector.tensor_mul(blk(P2, CL), blk(A, CL), pbc(t2, l0, CL))       # TA2 (P2 dead)
            nc.vector.tensor_mul(blk(AB2, CL), blk(Bt, CL), pbc(m2t, l0, CL))    # TB2 (AB2 dead)
            nc.vector.tensor_scalar(PR[:, :cw], P2[:, :cw], float(INV2PI), float(MAGIC), ALU.mult, ALU.add)  # Ft (PR dead)
            nc.vector.tensor_scalar(PP[:, :cw], PR[:, :cw], float(MAGIC), None, ALU.subtract)                # KK (PP dead)
            nc.vector.scalar_tensor_tensor(P2[:, :cw], PP[:, :cw], float(-2.0 * np.pi), P2[:, :cw], ALU.mult, ALU.add)  # TA2r in place
            # cos(x) = sin(pi/2 - |x|), valid for |x| <= pi
            nc.scalar.activation(CI0[:, :cw], P2[:, :cw], AF.Abs)                # |TA2r| (CI0 dead)
            nc.vector.tensor_scalar(RDIF[:, :cw], CI0[:, :cw], -1.0, float(PIO2), ALU.mult, ALU.add)  # CARG (RDIF dead)
            iEE = nc.scalar.activation(tB16[:, :cw], AB2[:, :cw], AF.Exp)        # EE fp16 (tB dead)
            iS2 = nc.scalar.activation(tA16[:, :cw], P2[:, :cw], AF.Sin)         # S2T fp16 (tA dead)
            bass._add_dep_helper(iS2.ins, iEE.ins, sync=False, reason="act-table-order")
            iC2 = nc.scalar.activation(WI16[:, :cw], RDIF[:, :cw], AF.Sin)       # C2T fp16 (WI dead)
            bass._add_dep_helper(iC2.ins, iEE.ins, sync=False, reason="act-table-order")
            prev_sin = iC2
            nc.vector.tensor_mul(ERE[:, :cw], tB16[:, :cw], WI16[:, :cw])        # ERE = EE*C2T
            nc.vector.tensor_mul(eimv[:, :, 0, :], blk(tB16, CL), blk(tA16, CL))  # +EIM = EE*S2T
            nc.scalar.activation(eimv[:, :, 1, :], eimv[:, :, 0, :], AF.Copy, bias=0.0, scale=-1.0)

            # ---- sequential update over layers (descending), fp16 ---------
            for j in range(CL - 1, -1, -1):
                sl = slice(j * QF, (j + 1) * QF)
                sl2 = slice(j * 2 * QF, (j + 1) * 2 * QF)
                U = ucur; Vq = unew
                U22 = U[:].rearrange("p (a b n) -> p a b n", a=2, b=2)
                u_rot = U22[:, ::-1, :, :]          # [u1r,u1i,u0r,u0i]
                u_rev = U22[:, ::-1, ::-1, :]       # [u1i,u1r,u0i,u0r]
                rre4 = RRE[:, sl].rearrange("p (a b n) -> p a b n", a=1, b=1).broadcast_to([P, 2, 2, QF])
                rim4 = RIMPM[:, sl2].rearrange("p (a t n) -> p a t n", a=1, t=2).broadcast_to([P, 2, 2, QF])
                nc.vector.tensor_mul(A4[:].rearrange("p (a b n) -> p a b n", a=2, b=2), u_rot, rre4)
                nc.vector.tensor_mul(B4[:].rearrange("p (a b n) -> p a b n", a=2, b=2), u_rev, rim4)
                nc.vector.tensor_add(T4[:], U[:], A4[:])
                nc.vector.tensor_add(Vq[:], T4[:], B4[:])   # [nu0r, nu0i, p1r, p1i]
                p1 = Vq[:, 2 * QF:4 * QF]
                p12 = p1.rearrange("p (b n) -> p b n", b=2)
                p1sw = p12[:, ::-1, :]
                ere2 = ERE[:, sl].rearrange("p (b n) -> p b n", b=1).broadcast_to([P, 2, QF])
                nc.vector.tensor_mul(E4[:].rearrange("p (b n) -> p b n", b=2), p12, ere2)
                nc.vector.tensor_mul(E5[:].rearrange("p (b n) -> p b n", b=2), p1sw,
                                     EIMPM[:, sl2].rearrange("p (b n) -> p b n", b=2))
                nc.vector.tensor_add(Vq[:, 2 * QF:4 * QF], E4[:], E5[:])
                ucur, unew = unew, ucur

        # ---- epilogue: out = |u1/u0|^2 -----------------------------------
        u0r, u0i = ucur[:, 0:QF], ucur[:, QF:2 * QF]
        u1r, u1i = ucur[:, 2 * QF:3 * QF], ucur[:, 3 * QF:4 * QF]
        d1 = tl("q1", [P, QF]); d2 = tl("q2", [P, QF]); d3 = tl("q3", [P, QF])
        nc.scalar.activation(d1[:], u0r, AF.Square)
        nc.scalar.activation(d2[:], u0i, AF.Square)
        nc.vector.tensor_add(d1[:], d1[:], d2[:])      # |u0|^2
        nc.scalar.activation(d2[:], d1[:], AF.Ln)
        nc.scalar.activation(d1[:], d2[:], AF.Exp, bias=0.0, scale=-1.0)  # 1/|u0|^2
        nc.vector.tensor_mul(d2[:], u1r, u0r)
        nc.vector.tensor_mul(d3[:], u1i, u0i)
        nc.vector.tensor_add(d2[:], d2[:], d3[:])
        nc.vector.tensor_mul(d2[:], d2[:], d1[:])      # qr
        OUT = tl("OUT", [P, QF])
        nc.vector.tensor_mul(d3[:], u1i, u0r)
        qi2 = tl("q4", [P, QF])
        nc.vector.tensor_mul(qi2[:], u1r, u0i)
        nc.vector.tensor_sub(d3[:], d3[:], qi2[:])
        nc.vector.tensor_mul(d3[:], d3[:], d1[:])      # qi
        nc.scalar.activation(d2[:], d2[:], AF.Square)
        nc.scalar.activation(d3[:], d3[:], AF.Square)
        nc.vector.tensor_add(OUT[:], d2[:], d3[:])
        nc.sync.dma_start(d_out[:], OUT[:])

    return nc


_NC_CACHE = None


def _get_nc():
    global _NC_CACHE
    if _NC_CACHE is None:
        _NC_CACHE = _build_kernel()
    return _NC_CACHE


def _prep_core_inputs(q, thickness, roughness, sld):
    """Host-side O(B*(Q+L)) prep; returns per-core input dicts."""
    q = q.astype(f32); th = thickness.astype(f32)
    rg = roughness.astype(f32); sld = sld.astype(f32)
    amb = sld[:, 0:1]
    negc64 = -(4.0 * math.pi * 1e-6) * (sld.astype(np.float64) - amb.astype(np.float64))
    negc = negc64.astype(f32)                       # [B, L+1]
    s2m = (-2.0 * rg * rg).astype(f32)
    s2p = (2.0 * rg * rg).astype(np.float16)
    t2 = (2.0 * th).astype(f32)
    m2t = (-2.0 * th).astype(f32)
    qq = ((q * f32(0.5)) ** 2).astype(f32)          # [B, Q]

    def rep4(arr):  # [BL, K] -> [128, K] (each row repeated 4x)
        return np.repeat(arr, 4, axis=0).copy()

    in_maps = []
    for c in range(NCORES):
        bs = slice(c * BL, (c + 1) * BL)
        in_maps.append({
            "qq": qq[bs].reshape(P, QF).copy(),
            "negc": rep4(negc[bs]),
            "s2m": rep4(s2m[bs]),
            "s2p": rep4(s2p[bs]),
            "t2": rep4(t2[bs]),
            "m2t": rep4(m2t[bs]),
        })
    return in_maps


def run(q, thickness, roughness, sld, trace=False, **trace_kwargs):
    from concourse.bass_utils import run_bass_kernel_spmd
    nc = _get_nc()
    in_maps = _prep_core_inputs(q, thickness, roughness, sld)
    res = run_bass_kernel_spmd(nc, in_maps, core_ids=list(range(NCORES)),
                               trace=trace, **trace_kwargs)
    out = np.empty((B, Q), f32)
    for c in range(NCORES):
        out[c * BL:(c + 1) * BL] = res.results[c]["out"].reshape(BL, Q)
    return out, res


def kernel(q, thickness, roughness, sld):
    out, _ = run(q, thickness, roughness, sld)
    return out


# revision 8
# speedup vs baseline: 1.3597x; 1.0326x over previous
"""Abeles matrix (neutron reflectivity) kernel for 8 Trainium2 NeuronCores.

Algorithm (per point (b,q), L=64 layers):
  k_l = A_l - i*B_l with A = sqrt((R+|x|)/2) branch-stable, B = (Y/2)/A,
        x = (q/2)^2 - 4pi*(sld_l - sld_0)*1e-6, Y = 4pi*1e-9, R = sqrt(x^2+Y^2)
  r_l = (RDIF + 2i*CI0)/|S|^2 * exp(-2 s^2 PR) * (1 + i*WI)   [small-angle phase,
        WI = 2 s^2 PI <= ~0.02 rad, rel err <= WI^2/2 ~ 2e-4]
  E_l = exp(-2 t B) * (cos(2 t A) - i sin(2 t A)),  cos(x) = sin(pi/2 - |x|)
  scaled transfer recurrence (fp16): u0' = u0 + r*u1 ; u1' = E*(r*u0 + u1)
  out = |u1/u0|^2

Sharding: pure data-parallel over batch, 32 rows of B=256 per core.
Per-core layout: 128 partitions = 32 b x 4 q-groups, 128 free = q within group.
All elementwise on DVE (fp16 2x where precision allows), transcendentals on ACT.
"""
import sys
sys.path.insert(0, "/opt/trn_rl_repo")
import math
import numpy as np

import concourse.bass as bass
import concourse.mybir as mybir
from concourse import tile
from contextlib import ExitStack

AF = mybir.ActivationFunctionType
ALU = mybir.AluOpType
F32 = mybir.dt.float32
F16 = mybir.dt.float16
f32 = np.float32

B, Q, L = 256, 512, 64
NCORES = 8
BL = B // NCORES           # 32 batch rows per core
P = 128                    # partitions
QF = 128                   # q elements per partition
CHUNK = 13                 # layers per chunk (5 chunks: 13,13,13,13,12)

YMAG = 4.0 * math.pi * 1e-9
Y2 = f32(YMAG * YMAG)
LNHALFY = f32(np.log(YMAG / 2.0))
PIO2 = f32(np.pi / 2.0)
INV2PI = f32(1.0 / (2.0 * np.pi))
MAGIC = f32(1.5 * 2.0 ** 23)

# ---------------------------------------------------------------------------
# Toolchain workarounds for this walrus build:
# 1) InstDrain cannot carry sem waits -> re-emit as sync-engine wait_ge's.
# 2) TensorScalarPtr / Activation-with-AP-scale / CopyPredicated cannot carry
#    sem waits -> strip them onto same-engine wait_ge carrier instructions.
# ---------------------------------------------------------------------------
_PATCHED = False


def _install_patches():
    global _PATCHED
    if _PATCHED:
        return
    _PATCHED = True

    def _handles(tc):
        hm = {}
        for h in tc.sems.allocated().values():
            hm[h.name] = h
        return hm

    def _drain_and_barrier(self, tick_clock, wait_clock):
        nc = self.nc
        drain_inst = nc.sync.drain()
        wait_clock.add_sem_waits(
            drain_inst.ins, tile.ScopedClock({None: tick_clock.global_clock})
        )
        ii = drain_inst.ins
        si = ii.sync_info
        waits = list(si.on_wait) if si is not None else []
        if waits:
            ii.sync_info = mybir.SyncInfo(on_wait=[], on_update=list(si.on_update))
            hm = _handles(self)
            for w in waits:
                h = hm.get(w.ant_name)
                assert h is not None and w.wait_mode == "sem-ge-imm"
                nc.sync.wait_ge(h, w.wait_value)
        nc.all_engine_barrier()
        assert self.sems is not None
        popped = nc._tile_sem_poison_stack.pop()
        assert popped is self._sem_poison
        nc.clear_and_free_semaphores(list(self.sems.allocated().values()))
        nc.all_engine_barrier()

    tile.TileContext._drain_and_barrier = _drain_and_barrier

    _orig_commit = tile.TileContext._commit_instruction

    _KEEP1 = (mybir.InstTensorLoad, mybir.InstTensorSave, mybir.InstTensorCopy,
              mybir.InstTensorTensor)

    def _simple_aps(inst):
        # >2-dim APs lower to the S3S3D3-style structs with no wait slots
        try:
            for a in list(inst.ins) + list(inst.outs):
                ap = getattr(a, "ap", None)
                if ap is not None and len(ap) > 2:
                    return False
        except Exception:
            return False
        return True

    def _commit_instruction(self, inst, lazy_reg_writes=True):
        si = getattr(inst, "sync_info", None)
        if si is not None and si.on_wait:
            waits = list(si.on_wait)
            keep = []
            if isinstance(inst, _KEEP1) and _simple_aps(inst):
                # these structs tolerate one wait; strip the rest
                keep = waits[:1]
                waits = waits[1:]
            if waits:
                inst.sync_info = mybir.SyncInfo(on_wait=keep, on_update=list(si.on_update))
                hm = _handles(self)
                eng = self.nc.engines[inst.engine]
                for i in range(0, len(waits), 2):
                    grp = waits[i:i + 2]
                    h = hm.get(grp[0].ant_name)
                    assert h is not None and grp[0].wait_mode == "sem-ge-imm", grp
                    carrier = eng.wait_ge(h, grp[0].wait_value)
                    if len(grp) > 1:
                        csi = carrier.ins.sync_info
                        carrier.ins.sync_info = mybir.SyncInfo(
                            on_wait=list(grp),
                            on_update=list(csi.on_update) if csi else [])
        return _orig_commit(self, inst, lazy_reg_writes)

    tile.TileContext._commit_instruction = _commit_instruction


# ---------------------------------------------------------------------------
# Kernel builder (one NeuronCore program; SPMD across 8 cores)
# ---------------------------------------------------------------------------

def _build_kernel():
    _install_patches()
    nc = bass.Bass()

    d_qq = nc.declare_dram_parameter("qq", [P, QF], F32, isOutput=False)
    d_negc = nc.declare_dram_parameter("negc", [P, L + 1], F32, isOutput=False)
    d_s2m = nc.declare_dram_parameter("s2m", [P, L], F32, isOutput=False)
    d_s2p = nc.declare_dram_parameter("s2p", [P, L], F16, isOutput=False)
    d_t2 = nc.declare_dram_parameter("t2", [P, L], F32, isOutput=False)
    d_m2t = nc.declare_dram_parameter("m2t", [P, L], F32, isOutput=False)
    d_out = nc.declare_dram_parameter("out", [P, QF], F32, isOutput=True)

    with tile.TileContext(nc) as tc, ExitStack() as ctx:
        pool = ctx.enter_context(tc.tile_pool(name="sb", bufs=1))

        def tl(name, shape, dtype=F32, bufs=1):
            return pool.tile(shape, dtype, tag=name, name=name, bufs=bufs)

        # persistent inputs
        qq = tl("qq", [P, QF])
        negc = tl("negc", [P, L + 1])
        s2m = tl("s2m", [P, L])
        s2p = tl("s2p", [P, L], F16)
        t2 = tl("t2", [P, L])
        m2t = tl("m2t", [P, L])
        nc.sync.dma_start(qq[:], d_qq[:])
        nc.sync.dma_start(negc[:], d_negc[:])
        nc.sync.dma_start(s2m[:], d_s2m[:])
        nc.sync.dma_start(s2p[:], d_s2p[:])
        nc.sync.dma_start(t2[:], d_t2[:])
        nc.sync.dma_start(m2t[:], d_m2t[:])

        # constant bias vectors
        y2b = tl("y2b", [P, 1]);  nc.gpsimd.memset(y2b[:], float(Y2))
        lnhy = tl("lnhy", [P, 1]); nc.gpsimd.memset(lnhy[:], float(LNHALFY))

        # u state (ping-pong quads: blocks [u0r | u0i | u1r | u1i]), fp16
        ucur = tl("uq_a", [P, 4 * QF], F16)
        unew = tl("uq_b", [P, 4 * QF], F16)
        nc.gpsimd.memset(ucur[:, 0:QF], 1.0)
        nc.gpsimd.memset(ucur[:, QF:], 0.0)
        A4 = tl("A4", [P, 4 * QF], F16); B4 = tl("B4", [P, 4 * QF], F16)
        T4 = tl("T4", [P, 4 * QF], F16)
        E4 = tl("E4", [P, 2 * QF], F16); E5 = tl("E5", [P, 2 * QF], F16)

        def pbc(param, lv0, n_l):
            # [P, n_l, QF] view of param[:, lv0:lv0+n_l], value broadcast over q
            return param[:, lv0:lv0 + n_l].rearrange("p (l n) -> p l n", n=1).broadcast_to([P, n_l, QF])

        def blk(t_, n_l):
            return t_[:, :n_l * QF].rearrange("p (l n) -> p l n", n=QF)

        prev_sin = None          # last trig-set op of previous chunk
        CW = CHUNK * QF          # max chunk width (layers)
        CW1 = (CHUNK + 1) * QF   # max chunk width (levels)
        starts = list(range(0, L, CHUNK))
        chunks = [(s0_, min(CHUNK, L - s0_)) for s0_ in starts]

        for l0, CL in reversed(chunks):
            cw = CL * QF
            cw1 = (CL + 1) * QF
            # ---- k levels: X, MSK, R, A, B --------------------------------
            X = tl("X", [P, CW1])
            MSK = tl("MSK", [P, CW1], mybir.dt.uint32)
            R = tl("Rr", [P, CW1])
            SQU = tl("SQU", [P, CW1])   # SQ -> G
            UHB = tl("UHB", [P, CW1])   # AX -> H -> B
            UA = tl("UA", [P, CW1])     # U -> A
            T = tl("T", [P, CW1])
            G = SQU                      # SQ dead once R is computed
            qqbc = qq[:].rearrange("p (l n) -> p l n", l=1).broadcast_to([P, CL + 1, QF])
            nc.vector.tensor_add(blk(X, CL + 1), qqbc, pbc(negc, l0, CL + 1))
            nc.vector.tensor_scalar(MSK[:, :cw1], X[:, :cw1], 0.0, None, ALU.is_ge)
            iSQ = nc.scalar.activation(SQU[:, :cw1], X[:, :cw1], AF.Square)             # SQ = X^2
            if prev_sin is not None:
                bass._add_dep_helper(iSQ.ins, prev_sin.ins, sync=False, reason="act-table-order")
            nc.scalar.activation(UHB[:, :cw1], X[:, :cw1], AF.Abs)                      # AX = |X|
            nc.scalar.activation(R[:, :cw1], SQU[:, :cw1], AF.Sqrt, bias=y2b[:])        # R = sqrt(X^2+Y2)
            nc.vector.tensor_add(UA[:, :cw1], R[:, :cw1], UHB[:, :cw1])                 # U = R + AX
            iT = nc.scalar.activation(T[:, :cw1], UA[:, :cw1], AF.Sqrt, bias=0.0, scale=0.5)  # T = sqrt(U/2)
            iH = nc.scalar.activation(UHB[:, :cw1], UA[:, :cw1], AF.Ln, bias=0.0, scale=0.5)   # H = ln(U/2)
            bass._add_dep_helper(iH.ins, iT.ins, sync=False, reason="act-table-order")
            nc.scalar.activation(G[:, :cw1], UHB[:, :cw1], AF.Exp, bias=lnhy[:], scale=-0.5)  # G=(Y/2)/T
            A = UA; Bt = UHB
            nc.scalar.copy(A[:, :cw1], G[:, :cw1])
            nc.vector.copy_predicated(A[:, :cw1], MSK[:, :cw1], T[:, :cw1])   # A = x>=0 ? T : G
            nc.scalar.copy(Bt[:, :cw1], T[:, :cw1])
            nc.vector.copy_predicated(Bt[:, :cw1], MSK[:, :cw1], G[:, :cw1])  # B = x>=0 ? G : T

            ac, an = A[:, 0:cw], A[:, QF:cw1]
            bc, bn = Bt[:, 0:cw], Bt[:, QF:cw1]
            rc, rn = R[:, 0:cw], R[:, QF:cw1]

            # ---- layer quantities (fp32 where cancellation matters) -------
            P1 = tl("P1", [P, CW])              # P1 -> LNM
            P2 = tl("P2", [P, CW])              # P2 -> TA2
            AB1 = tl("AB1", [P, CW])            # AB1 -> ARG
            AB2 = tl("AB2", [P, CW])            # AB2 -> TB2
            PR = tl("PR", [P, CW])              # PR -> Ft
            PP = tl("PP", [P, CW])              # PP -> KK
            CI0 = tl("CI0", [P, CW])            # CI0 -> |TA2r|
            RSUM = tl("RSUM", [P, CW])
            RDIF = tl("RDIF", [P, CW])          # RDIF -> CARG
            MAG = tl("MAG", [P, CW])            # MAG -> RR
            PI16 = tl("PI16", [P, CW], F16)
            RATRE = tl("RATRE", [P, CW], F16)
            RATIM = tl("RATIM", [P, CW], F16)
            WI16 = tl("WI16", [P, CW], F16)     # WI -> C2T
            tA16 = tl("tA16", [P, CW], F16)     # tA -> S2T
            tB16 = tl("tB16", [P, CW], F16)     # tB -> EE

            nc.vector.tensor_mul(P1[:, :cw], ac, an)
            nc.vector.tensor_mul(P2[:, :cw], bc, bn)
            nc.vector.tensor_mul(AB1[:, :cw], ac, bn)
            nc.vector.tensor_mul(AB2[:, :cw], bc, an)
            nc.vector.tensor_add(RSUM[:, :cw], rc, rn)
            nc.vector.tensor_sub(RDIF[:, :cw], rc, rn)
            nc.vector.tensor_sub(PR[:, :cw], P1[:, :cw], P2[:, :cw])
            nc.vector.tensor_add(PP[:, :cw], P1[:, :cw], P2[:, :cw])
            nc.vector.tensor_sub(CI0[:, :cw], AB1[:, :cw], AB2[:, :cw])
            nc.vector.tensor_add(PI16[:, :cw], AB1[:, :cw], AB2[:, :cw])
            nc.vector.scalar_tensor_tensor(MAG[:, :cw], PP[:, :cw], 2.0, RSUM[:, :cw], ALU.mult, ALU.add)
            nc.scalar.activation(P1[:, :cw], MAG[:, :cw], AF.Ln)                 # LNM (P1 dead)
            nc.vector.tensor_mul(blk(AB1, CL), blk(PR, CL), pbc(s2m, l0, CL))    # ARG = PR*(-2s^2)
            nc.vector.tensor_sub(AB1[:, :cw], AB1[:, :cw], P1[:, :cw])           # ARG -= LNM
            nc.scalar.activation(MAG[:, :cw], AB1[:, :cw], AF.Exp)               # RR (MAG dead)
            nc.vector.tensor_mul(RATRE[:, :cw], RDIF[:, :cw], MAG[:, :cw])
            nc.vector.scalar_tensor_tensor(RATIM[:, :cw], CI0[:, :cw], 2.0, MAG[:, :cw], ALU.mult, ALU.mult)
            nc.vector.tensor_mul(blk(WI16, CL), blk(PI16, CL), pbc(s2p, l0, CL))  # WI = PI*2s^2
            # r = (RATRE + i RATIM)(1 + i WI): small-angle phase
            RRE = tl("RRE", [P, CW], F16, bufs=2)
            RIMPM = tl("RIMPM", [P, 2 * CW], F16, bufs=2)
            ERE = tl("ERE", [P, CW], F16, bufs=2)
            EIMPM = tl("EIMPM", [P, 2 * CW], F16, bufs=2)
            rimv = RIMPM[:, :2 * cw].rearrange("p (l t n) -> p l t n", t=2, n=QF)
            eimv = EIMPM[:, :2 * cw].rearrange("p (l t n) -> p l t n", t=2, n=QF)
            nc.vector.tensor_mul(tA16[:, :cw], RATIM[:, :cw], WI16[:, :cw])
            nc.vector.tensor_sub(RRE[:, :cw], RATRE[:, :cw], tA16[:, :cw])
            nc.vector.tensor_mul(tB16[:, :cw], RATRE[:, :cw], WI16[:, :cw])
            nc.vector.tensor_add(rimv[:, :, 1, :], blk(tB16, CL), blk(RATIM, CL))
            nc.scalar.activation(rimv[:, :, 0, :], rimv[:, :, 1, :], AF.Copy, bias=0.0, scale=-1.0)
            # ---- E = exp(-2tB)*(cos 2tA - i sin 2tA) ----------------------
            nc.vector.tensor_mul(blk(P2, CL), blk(A, CL), pbc(t2, l0, CL))       # TA2 (P2 dead)
            nc.vector.tensor_mul(blk(AB2, CL), blk(Bt, CL), pbc(m2t, l0, CL))    # TB2 (AB2 dead)
            nc.vector.tensor_scalar(PR[:, :cw], P2[:, :cw], float(INV2PI), float(MAGIC), ALU.mult, ALU.add)  # Ft (PR dead)
            nc.vector.tensor_scalar(PP[:, :cw], PR[:, :cw], float(MAGIC), None, ALU.subtract)                # KK (PP dead)
            nc.vector.scalar_tensor_tensor(P2[:, :cw], PP[:, :cw], float(-2.0 * np.pi), P2[:, :cw], ALU.mult, ALU.add)  # TA2r in place
            # cos(x) = sin(pi/2 - |x|), valid for |x| <= pi
            nc.scalar.activation(CI0[:, :cw], P2[:, :cw], AF.Abs)                # |TA2r| (CI0 dead)
            nc.vector.tensor_scalar(RDIF[:, :cw], CI0[:, :cw], -1.0, float(PIO2), ALU.mult, ALU.add)  # CARG (RDIF dead)
            iEE = nc.scalar.activation(tB16[:, :cw], AB2[:, :cw], AF.Exp)        # EE fp16 (tB dead)
            iS2 = nc.scalar.activation(tA16[:, :cw], P2[:, :cw], AF.Sin)         # S2T fp16 (tA dead)
            bass._add_dep_helper(iS2.ins, iEE.ins, sync=False, reason="act-table-order")
            iC2 = nc.scalar.activation(WI16[:, :cw], RDIF[:, :cw], AF.Sin)       # C2T fp16 (WI dead)
            bass._add_dep_helper(iC2.ins, iEE.ins, sync=False, reason="act-table-order")
            prev_sin = iC2
            nc.vector.tensor_mul(ERE[:, :cw], tB16[:, :cw], WI16[:, :cw])        # ERE = EE*C2T
            nc.vector.tensor_mul(eimv[:, :, 0, :], blk(tB16, CL), blk(tA16, CL))  # +EIM = EE*S2T
            nc.scalar.activation(eimv[:, :, 1, :], eimv[:, :, 0, :], AF.Copy, bias=0.0, scale=-1.0)

            # ---- sequential update over layers (descending), fp16 ---------
            for j in range(CL - 1, -1, -1):
                sl = slice(j * QF, (j + 1) * QF)
                sl2 = slice(j * 2 * QF, (j + 1) * 2 * QF)
                U = ucur; Vq = unew
                U22 = U[:].rearrange("p (a b n) -> p a b n", a=2, b=2)
                u_rot = U22[:, ::-1, :, :]          # [u1r,u1i,u0r,u0i]
                u_rev = U22[:, ::-1, ::-1, :]       # [u1i,u1r,u0i,u0r]
                rre4 = RRE[:, sl].rearrange("p (a b n) -> p a b n", a=1, b=1).broadcast_to([P, 2, 2, QF])
                rim4 = RIMPM[:, sl2].rearrange("p (a t n) -> p a t n", a=1, t=2).broadcast_to([P, 2, 2, QF])
                nc.vector.tensor_mul(A4[:].rearrange("p (a b n) -> p a b n", a=2, b=2), u_rot, rre4)
                nc.vector.tensor_mul(B4[:].rearrange("p (a b n) -> p a b n", a=2, b=2), u_rev, rim4)
                nc.vector.tensor_add(T4[:], U[:], A4[:])
                nc.vector.tensor_add(Vq[:], T4[:], B4[:])   # [nu0r, nu0i, p1r, p1i]
                p1 = Vq[:, 2 * QF:4 * QF]
                p12 = p1.rearrange("p (b n) -> p b n", b=2)
                p1sw = p12[:, ::-1, :]
                ere2 = ERE[:, sl].rearrange("p (b n) -> p b n", b=1).broadcast_to([P, 2, QF])
                nc.vector.tensor_mul(E4[:].rearrange("p (b n) -> p b n", b=2), p12, ere2)
                nc.vector.tensor_mul(E5[:].rearrange("p (b n) -> p b n", b=2), p1sw,
                                     EIMPM[:, sl2].rearrange("p (b n) -> p b n", b=2))
                nc.vector.tensor_add(Vq[:, 2 * QF:4 * QF], E4[:], E5[:])
                ucur, unew = unew, ucur

        # ---- epilogue: out = |u1/u0|^2 -----------------------------------
        u0r, u0i = ucur[:, 0:QF], ucur[:, QF:2 * QF]
        u1r, u1i = ucur[:, 2 * QF:3 * QF], ucur[:, 3 * QF:4 * QF]
        d1 = tl("q1", [P, QF]); d2 = tl("q2", [P, QF]); d3 = tl("q3", [P, QF])
        nc.scalar.activation(d1[:], u0r, AF.Square)
        nc.scalar.activation(d2[:], u0i, AF.Square)
        nc.vector.tensor_add(d1[:], d1[:], d2[:])      # |u0|^2
        nc.scalar.activation(d2[:], d1[:], AF.Ln)
        nc.scalar.activation(d1[:], d2[:], AF.Exp, bias=0.0, scale=-1.0)  # 1/|u0|^2
        nc.vector.tensor_mul(d2[:], u1r, u0r)
        nc.vector.tensor_mul(d3[:], u1i, u0i)
        nc.vector.tensor_add(d2[:], d2[:], d3[:])
        nc.vector.tensor_mul(d2[:], d2[:], d1[:])      # qr
        OUT = tl("OUT", [P, QF])
        nc.vector.tensor_mul(d3[:], u1i, u0r)
        qi2 = tl("q4", [P, QF])
        nc.vector.tensor_mul(qi2[:], u1r, u0i)
        nc.vector.tensor_sub(d3[:], d3[:], qi2[:])
        nc.vector.tensor_mul(d3[:], d3[:], d1[:])      # qi
        nc.scalar.activation(d2[:], d2[:], AF.Square)
        nc.scalar.activation(d3[:], d3[:], AF.Square)
        nc.vector.tensor_add(OUT[:], d2[:], d3[:])
        nc.sync.dma_start(d_out[:], OUT[:])

    return nc


_NC_CACHE = None


def _get_nc():
    global _NC_CACHE
    if _NC_CACHE is None:
        _NC_CACHE = _build_kernel()
    return _NC_CACHE


def _prep_core_inputs(q, thickness, roughness, sld):
    """Host-side O(B*(Q+L)) prep; returns per-core input dicts."""
    q = q.astype(f32); th = thickness.astype(f32)
    rg = roughness.astype(f32); sld = sld.astype(f32)
    amb = sld[:, 0:1]
    negc64 = -(4.0 * math.pi * 1e-6) * (sld.astype(np.float64) - amb.astype(np.float64))
    negc = negc64.astype(f32)                       # [B, L+1]
    s2m = (-2.0 * rg * rg).astype(f32)
    s2p = (2.0 * rg * rg).astype(np.float16)
    t2 = (2.0 * th).astype(f32)
    m2t = (-2.0 * th).astype(f32)
    qq = ((q * f32(0.5)) ** 2).astype(f32)          # [B, Q]

    def rep4(arr):  # [BL, K] -> [128, K] (each row repeated 4x)
        return np.repeat(arr, 4, axis=0).copy()

    in_maps = []
    for c in range(NCORES):
        bs = slice(c * BL, (c + 1) * BL)
        in_maps.append({
            "qq": qq[bs].reshape(P, QF).copy(),
            "negc": rep4(negc[bs]),
            "s2m": rep4(s2m[bs]),
            "s2p": rep4(s2p[bs]),
            "t2": rep4(t2[bs]),
            "m2t": rep4(m2t[bs]),
        })
    return in_maps


def run(q, thickness, roughness, sld, trace=False, **trace_kwargs):
    from concourse.bass_utils import run_bass_kernel_spmd
    nc = _get_nc()
    in_maps = _prep_core_inputs(q, thickness, roughness, sld)
    res = run_bass_kernel_spmd(nc, in_maps, core_ids=list(range(NCORES)),
                               trace=trace, **trace_kwargs)
    out = np.empty((B, Q), f32)
    for c in range(NCORES):
        out[c * BL:(c + 1) * BL] = res.results[c]["out"].reshape(BL, Q)
    return out, res


def kernel(q, thickness, roughness, sld):
    out, _ = run(q, thickness, roughness, sld)
    return out
